# revision 6
# baseline (speedup 1.0000x reference)
"""Trainium2 Bass kernel for a 2-layer cross-encoder (CrossEncoder).

Model: B=2, NQ=NKV=2048, E=512, H=8 (d_head=64), MLP=2048, depth=2, fp32 I/O.

Sharding (8 cores, no collectives): core c handles batch b=c//4 and query
rows [qc*512, (qc+1)*512) with qc=c%4.  Each core computes the full KV
projections for its batch so every core produces its output slice
independently.

Numerics: projection/FFN/attn-value matmuls run in fp8e4m3 with the
DoubleRow perf mode (two 128-deep k-tiles contracted per instruction).
Weights are scaled x32 on the host so their 0.02-std values sit in e4m3's
normal range; every PSUM->SBUF pass descales.  QK^T scores stay bf16.
The residual stream, LayerNorm statistics and softmax normalization stay
fp32.  LN gamma/beta are folded into the projection weights on the host.
The softmax denominator comes free from a ones-column appended to V (the
un-normalized attn@V matmul also computes sum(exp) in column 64); the
attention output is scaled x4 via the replicate row to keep fp8 aoT in
e4m3's normal range.
"""

import numpy as np
import ml_dtypes

import concourse.bass as bass
import concourse.bacc as bacc
import concourse.mybir as mybir
import concourse.tile as tile
from concourse import bass_utils, masks
from contextlib import ExitStack

P = 128
E = 512
EC = E // P        # 4 chunks of the embedding dim
NQ = 512           # query rows per core
QC = NQ // P       # 4 query chunks
NKV = 2048
KC = NKV // P      # 16 key chunks of 128
KN = NKV // 512    # 4 key chunks of 512
H = 8
DH = 64
MLP = 2048
MC = MLP // P      # 16 mlp chunks of 128
L = 2
LN_EPS = 1e-5
F32 = mybir.dt.float32
BF16 = mybir.dt.bfloat16
FP8 = mybir.dt.float8e4
AF = mybir.ActivationFunctionType
ALU = mybir.AluOpType
DR = mybir.MatmulPerfMode.DoubleRow
SCALE = (E // H) ** -0.5
WS = 32.0          # host-side weight scale into fp8
AOS = 4.0          # aoT scale via replicate row

_CACHE = {}


def _build(use_bias):
    """Build the per-core Bass program (identical on all 8 cores)."""
    nc = bacc.Bacc("TRN2", target_bir_lowering=False, debug=False, num_devices=8)

    xq_d = nc.dram_tensor("xq", [NQ, E], F32, kind="ExternalInput").ap()
    xkv_d = nc.dram_tensor("xkv", [NKV, E], F32, kind="ExternalInput").ap()
    wd = []
    for l in range(L):
        wd.append({
            "wq": nc.dram_tensor(f"wq{l}", [P, EC * E], FP8, kind="ExternalInput").ap(),
            "wk": nc.dram_tensor(f"wk{l}", [P, EC * E], FP8, kind="ExternalInput").ap(),
            "wv": nc.dram_tensor(f"wv{l}", [P, EC * E], FP8, kind="ExternalInput").ap(),
            "wo": nc.dram_tensor(f"wo{l}", [P, EC * E], BF16, kind="ExternalInput").ap(),
            "w1": nc.dram_tensor(f"w1{l}", [P, EC * MLP], FP8, kind="ExternalInput").ap(),
            "w2": nc.dram_tensor(f"w2{l}", [P, MC * E], BF16, kind="ExternalInput").ap(),
        })
        if use_bias:
            wd[-1].update({
                "bq": nc.dram_tensor(f"bq{l}", [P, EC], F32, kind="ExternalInput").ap(),
                "bk": nc.dram_tensor(f"bk{l}", [P, EC], F32, kind="ExternalInput").ap(),
                "b1": nc.dram_tensor(f"b1{l}", [P, MC], F32, kind="ExternalInput").ap(),
                "bo": nc.dram_tensor(f"bo{l}", [P, E], F32, kind="ExternalInput").ap(),
                "b2": nc.dram_tensor(f"b2{l}", [P, E], F32, kind="ExternalInput").ap(),
            })
    y_d = nc.dram_tensor("y", [NQ, E], F32, kind="ExternalOutput").ap()

    with tile.TileContext(nc) as tc, ExitStack() as ctx:
        const_pool = ctx.enter_context(tc.tile_pool(name="const", bufs=1))
        ident = const_pool.tile([P, P], BF16)
        masks.make_identity(nc, ident)
        ones_row = const_pool.tile([1, DH], BF16)
        nc.gpsimd.memset(ones_row[:], AOS)
        eps_col = const_pool.tile([P, 1], F32)
        nc.gpsimd.memset(eps_col[:], LN_EPS)

        stats_pool = ctx.enter_context(tc.tile_pool(name="stats", bufs=12))

        def ln_tile(x_t, out_pool, out_name, apply_eng=None):
            """LayerNorm core (x - mu) * rsqrt(var + eps), fp32 in, bf16 out."""
            bnst = stats_pool.tile([P, 6], F32, name="bnst")
            nc.vector.bn_stats(bnst[:], x_t)
            bnag = stats_pool.tile([P, 2], F32, name="bnag")
            nc.vector.bn_aggr(bnag[:], bnst[:])
            sq = stats_pool.tile([P, 1], F32, name="sq")
            nc.scalar.activation(sq[:], bnag[:, 1:2], AF.Sqrt, bias=eps_col[:])
            rstd = stats_pool.tile([P, 1], F32, name="rstd")
            nc.vector.reciprocal(rstd[:], sq[:])
            h_t = out_pool.tile([P, E], BF16, name=out_name, bufs=4)
            (apply_eng or nc.gpsimd).tensor_scalar(
                h_t[:], x_t, bnag[:, 0:1], rstd[:], op0=ALU.subtract, op1=ALU.mult
            )
            return h_t

        # Residual stream: 4 fp32 tiles of [128, 512].
        xq_pool = ctx.enter_context(tc.tile_pool(name="xq", bufs=1))
        xq = []
        for i in range(QC):
            t = xq_pool.tile([P, E], F32, name=f"xq{i}", tag=f"xq{i}")
            nc.sync.dma_start(t[:], xq_d[i * P:(i + 1) * P, :])
            xq.append(t[:])

        # hkv^T: LN1-core of x_kv, transposed to [E, NKV], fp8.  ln1 g/b are
        # folded into the weights, so this is layer-independent.
        hkvT_pool = ctx.enter_context(tc.tile_pool(name="hkvT", bufs=1))
        hkvT = hkvT_pool.tile([P, EC, NKV], FP8, name="hkvT", tag="hkvT")

        # v_aug: per kv-chunk, per head: [64 v-cols | ones | zero pad] fp8.
        vaug_pool = ctx.enter_context(tc.tile_pool(name="vaug", bufs=1))
        v_aug = vaug_pool.tile([P, KC, H, DH + 1], BF16, name="v_aug", tag="v_aug")
        nc.gpsimd.memset(v_aug[:, :, :, DH:DH + 1], 1.0)

        # PSUM pools (8 banks total): pp 2 + ss 2x2 + att 2 = 8.
        pp_pool = ctx.enter_context(tc.tile_pool(name="pp", bufs=2, space="PSUM"))
        ss_pool = ctx.enter_context(tc.tile_pool(name="ss", bufs=2, space="PSUM"))
        att_pool = ctx.enter_context(tc.tile_pool(name="attp", bufs=2, space="PSUM"))

        def transpose_block(dst, src_block, copy_engine="vector"):
            """dst (fp8 SBUF slice) = src_block.T via PE (bf16) + copy."""
            pt = pp_pool.tile([P, E], F32, name="pp", tag="pp")
            ptb = pt[:].bitcast(BF16)[:, 0:P]
            nc.tensor.transpose(ptb, src_block, ident[:])
            if copy_engine == "scalar":
                nc.scalar.copy(dst, ptb)
            else:
                nc.vector.tensor_copy(dst, ptb)

        # Weight pools (bufs=2 -> next layer prefetches during current layer).
        wpool = ctx.enter_context(tc.tile_pool(name="w", bufs=2))

        def alloc_weights_crit(w):
            d = {}
            d["wq"] = wpool.tile([P, EC, E], FP8, name="wq_sb", tag="wq")
            nc.sync.dma_start(d["wq"][:], w["wq"].rearrange("p (c e) -> p c e", c=EC))
            d["wk"] = wpool.tile([P, EC, E], FP8, name="wk_sb", tag="wk")
            nc.sync.dma_start(d["wk"][:], w["wk"].rearrange("p (c e) -> p c e", c=EC))
            d["wv"] = wpool.tile([P, EC, E], FP8, name="wv_sb", tag="wv")
            nc.sync.dma_start(d["wv"][:], w["wv"].rearrange("p (c e) -> p c e", c=EC))
            if use_bias:
                d["bq"] = wpool.tile([P, EC], F32, name="bq_sb", tag="bq")
                nc.sync.dma_start(d["bq"][:], w["bq"])
                d["bk"] = wpool.tile([P, EC], F32, name="bk_sb", tag="bk")
                nc.sync.dma_start(d["bk"][:], w["bk"])
            return d

        def alloc_weights_rest(d, w):
            d["wo"] = wpool.tile([P, EC, E], BF16, name="wo_sb", tag="wo")
            nc.sync.dma_start(d["wo"][:], w["wo"].rearrange("p (c e) -> p c e", c=EC))
            d["w1"] = wpool.tile([P, EC, MLP], FP8, name="w1_sb", tag="w1", bufs=1)
            nc.sync.dma_start(d["w1"][:], w["w1"].rearrange("p (c e) -> p c e", c=EC))
            d["w2"] = wpool.tile([P, MC, E], BF16, name="w2_sb", tag="w2", bufs=1)
            nc.sync.dma_start(d["w2"][:], w["w2"].rearrange("p (c e) -> p c e", c=MC))
            if use_bias:
                d["b1"] = wpool.tile([P, MC], F32, name="b1_sb", tag="b1")
                nc.sync.dma_start(d["b1"][:], w["b1"])
                d["bo"] = wpool.tile([P, E], F32, name="bo_sb", tag="bo", bufs=1)
                nc.sync.dma_start(d["bo"][:], w["bo"])
                d["b2"] = wpool.tile([P, E], F32, name="b2_sb", tag="b2", bufs=1)
                nc.sync.dma_start(d["b2"][:], w["b2"])
            return d

        w0 = None
        with tc.tile_pool(name="xkv", bufs=4) as xkv_pool:
            for ib in range(KC // 2):
                if ib == 2:
                    w0 = alloc_weights_crit(wd[0])
                xkv_t = xkv_pool.tile([P, 2, E], F32, name="xkv_t", tag="xkv_t")
                nc.sync.dma_start(
                    xkv_t[:],
                    xkv_d[ib * 2 * P:(ib + 1) * 2 * P, :].rearrange(
                        "(i p) c -> p i c", p=P
                    ),
                )
                for sub in range(2):
                    i = 2 * ib + sub
                    hkv_t = ln_tile(
                        xkv_t[:, sub, :], xkv_pool, "hkv_t",
                        apply_eng=nc.gpsimd if i % 2 else nc.vector,
                    )
                    for e in range(EC):
                        transpose_block(
                            hkvT[:, e, i * P:(i + 1) * P],
                            hkv_t[:, e * P:(e + 1) * P],
                            "scalar" if (i + e) % 2 else "vector",
                        )

        w0 = alloc_weights_rest(w0, wd[0])

        # Work pools.
        work = ctx.enter_context(tc.tile_pool(name="work", bufs=1))
        big = ctx.enter_context(tc.tile_pool(name="big", bufs=1))
        ex_pool = ctx.enter_context(tc.tile_pool(name="ex", bufs=5))

        def psum_to_sbuf(out, ps, descale, bias, eng=None):
            """out = ps*descale (+bias col) on a vector engine (DVE default)."""
            eng = eng or nc.vector
            if use_bias and bias is not None:
                eng.tensor_scalar(out, ps, descale, bias, op0=ALU.mult, op1=ALU.add)
            else:
                eng.tensor_scalar(out, ps, descale, None, op0=ALU.mult)

        for l in range(L):
            if l == 0:
                wt = w0
            else:
                wt = alloc_weights_crit(wd[l])
                wt = alloc_weights_rest(wt, wd[l])

            # ---- LN1(x_q) and transpose -> hqT [E, NQ] fp8 ----
            hqT = work.tile([P, EC, NQ], FP8, name="hqT", tag="actT")
            for qc in range(QC):
                hq_t = ln_tile(xq[qc], work, "hq_t")
                for e in range(EC):
                    transpose_block(
                        hqT[:, e, qc * P:(qc + 1) * P],
                        hq_t[:, e * P:(e + 1) * P],
                        "scalar" if (qc + e) % 2 else "vector",
                    )

            # ---- q^T = wq^T @ hq^T (+ bq)  [E, NQ] bf16 ----
            qT = [
                work.tile([P, NQ], BF16, name=f"qT{m}", tag=f"qT{m}")
                for m in range(EC)
            ]
            for m in range(EC):
                ps = pp_pool.tile([P, E], F32, name="pp", tag="pp")
                for t in range(2):
                    nc.tensor.matmul(
                        ps[:],
                        wt["wq"][:, 2 * t:2 * t + 2, m * P:(m + 1) * P],
                        hqT[:, 2 * t:2 * t + 2, :],
                        start=(t == 0),
                        stop=(t == 1),
                        perf_mode=DR,
                    )
                psum_to_sbuf(qT[m][:], ps[:], 1.0 / WS,
                             wt["bq"][:, m:m + 1] if use_bias else None)

            # ---- k^T = wk^T @ hkv^T (+ bk)  [E, NKV] bf16 ----
            kT = [
                big.tile([P, NKV], BF16, name=f"kT{m}", tag=f"kT{m}", bufs=2)
                for m in range(EC)
            ]
            for m in range(EC):
                for n in range(KN):
                    ps = pp_pool.tile([P, E], F32, name="pp", tag="pp")
                    for t in range(2):
                        nc.tensor.matmul(
                            ps[:],
                            wt["wk"][:, 2 * t:2 * t + 2, m * P:(m + 1) * P],
                            hkvT[:, 2 * t:2 * t + 2, n * 512:(n + 1) * 512],
                            start=(t == 0),
                            stop=(t == 1),
                            perf_mode=DR,
                        )
                    psum_to_sbuf(kT[m][:, n * 512:(n + 1) * 512], ps[:], 1.0 / WS,
                                 wt["bk"][:, m:m + 1] if use_bias else None)

            # ---- v = hkv @ wv  [NKV, E] -> v_aug fp8 ----
            for m in range(KC):
                ps = pp_pool.tile([P, E], F32, name="pp", tag="pp")
                for t in range(2):
                    nc.tensor.matmul(
                        ps[:],
                        hkvT[:, 2 * t:2 * t + 2, m * P:(m + 1) * P],
                        wt["wv"][:, 2 * t:2 * t + 2, :],
                        start=(t == 0),
                        stop=(t == 1),
                        perf_mode=DR,
                    )
                nc.vector.tensor_scalar(
                    v_aug[:, m, :, 0:DH],
                    ps[:].rearrange("p (h d) -> p h d", h=H),
                    1.0 / WS,
                    None,
                    op0=ALU.mult,
                )

            # ---- attention, head by head; writes aoT (x AOS) fp8 ----
            aoT = work.tile([P, EC, NQ], BF16, name="aoT", tag="aoT")
            for h in range(H):
                fh, r0 = h // 2, (h % 2) * DH
                # rows 0..63 = unnormalized attn@v (x WS); row 64 = sum(exp).
                ps_oT = att_pool.tile([P, E], F32, name="ps_oT", tag="att")
                for g in range(KC // 2):
                    ps_s = ss_pool.tile([P, 2, NQ], F32, name="ps_s", tag="ss")
                    for sub in range(2):
                        m = 2 * g + sub
                        nc.tensor.matmul(
                            ps_s[:, sub, :],
                            kT[fh][r0:r0 + DH, m * P:(m + 1) * P],
                            qT[fh][r0:r0 + DH, :],
                            start=True,
                            stop=True,
                        )
                    ex = ex_pool.tile([P, 2, NQ], BF16, name="ex", tag="ex")
                    nc.scalar.activation(ex[:], ps_s[:], AF.Exp, scale=SCALE)
                    for sub in range(2):
                        m = 2 * g + sub
                        nc.tensor.matmul(
                            ps_oT[0:DH + 1, :],
                            v_aug[:, m, h, :],
                            ex[:, sub, :],
                            start=(m == 0),
                            stop=(m == KC - 1),
                        )
                # normalize: aoT rows = AOS * unnorm / denom.
                rcp = stats_pool.tile([1, NQ], BF16, name="rcp", bufs=2)
                with nc.allow_low_precision(reason="f32r recip row for PE replicate"):
                    nc.vector.reciprocal(rcp[:], ps_oT[DH:DH + 1, :])
                ps_rep = pp_pool.tile([P, E], F32, name="pp", tag="pp")
                nc.tensor.matmul(
                    ps_rep[0:DH, :],
                    ones_row[:],
                    rcp[:],
                    start=True,
                    stop=True,
                )
                u_sb = work.tile([P, NQ], BF16, name="u_sb", bufs=2)
                nc.vector.tensor_copy(u_sb[0:DH, :], ps_oT[0:DH, :])
                nc.vector.tensor_mul(
                    aoT[r0:r0 + DH, fh, :], u_sb[0:DH, :], ps_rep[0:DH, :]
                )

            # ---- out-proj, residual ----
            for qc in range(QC):
                ps = pp_pool.tile([P, E], F32, name="pp", tag="pp")
                for kk in range(EC):
                    nc.tensor.matmul(
                        ps[:],
                        aoT[:, kk, qc * P:(qc + 1) * P],
                        wt["wo"][:, kk, :],
                        start=(kk == 0),
                        stop=(kk == EC - 1),
                    )
                if use_bias:
                    nc.vector.tensor_add(ps[:], ps[:], wt["bo"][:])
                nc.vector.tensor_add(xq[qc], xq[qc], ps[:])

            # ---- LN2 + transpose -> h2T fp8 ----
            h2T = work.tile([P, EC, NQ], FP8, name="h2T", tag="actT2")
            for qc in range(QC):
                h2_t = ln_tile(xq[qc], work, "hq_t")
                for e in range(EC):
                    transpose_block(
                        h2T[:, e, qc * P:(qc + 1) * P],
                        h2_t[:, e * P:(e + 1) * P],
                        "scalar" if (qc + e) % 2 else "vector",
                    )

            # ---- FFN1: g^T = gelu((w1^T @ h2^T)/WS + b1)  [MLP, NQ] fp8 ----
            gT = big.tile([P, MC, NQ], BF16, name="gT", tag="gT")
            for mg in range(MC // 2):
                ps = ss_pool.tile([P, 2, NQ], F32, name="ps_f", tag="ss")
                for j in range(2):
                    m = 2 * mg + j
                    for t in range(2):
                        nc.tensor.matmul(
                            ps[:, j, :],
                            wt["w1"][:, 2 * t:2 * t + 2, m * P:(m + 1) * P],
                            h2T[:, 2 * t:2 * t + 2, :],
                            start=(t == 0),
                            stop=(t == 1),
                            perf_mode=DR,
                        )
                if use_bias:
                    for j in range(2):
                        m = 2 * mg + j
                        nc.scalar.activation(
                            gT[:, m, :], ps[:, j, :], AF.Gelu,
                            bias=wt["b1"][:, m:m + 1], scale=1.0 / WS,
                        )
                else:
                    nc.scalar.activation(
                        gT[:, 2 * mg:2 * mg + 2, :], ps[:], AF.Gelu, scale=1.0 / WS
                    )

            # ---- FFN2 + residual ----
            for qc in range(QC):
                ps = pp_pool.tile([P, E], F32, name="pp", tag="pp")
                for g in range(MC):
                    nc.tensor.matmul(
                        ps[:],
                        gT[:, g, qc * P:(qc + 1) * P],
                        wt["w2"][:, g, :],
                        start=(g == 0),
                        stop=(g == MC - 1),
                    )
                if use_bias:
                    nc.vector.tensor_add(ps[:], ps[:], wt["b2"][:])
                nc.vector.tensor_add(xq[qc], xq[qc], ps[:])

        for qc in range(QC):
            nc.sync.dma_start(y_d[qc * P:(qc + 1) * P, :], xq[qc])

    nc.compile()
    return nc


def get_nc(use_bias=False):
    key = ("nc", use_bias)
    if key not in _CACHE:
        _CACHE[key] = _build(use_bias)
    return _CACHE[key]


def _rearr(w, k):
    """[k*128, C] row-major -> [128, k*C] with free layout (chunk, col)."""
    c = w.shape[1]
    return np.ascontiguousarray(
        w.reshape(k, P, c).transpose(1, 0, 2).reshape(P, k * c)
    )


def _cols(v):
    """[k*128] -> [128, k]: column m holds v[m*128:(m+1)*128]."""
    k = v.shape[0] // P
    return np.ascontiguousarray(v.reshape(k, P).T)


def _fp8(a):
    return np.asarray(a, dtype=np.float32).astype(ml_dtypes.float8_e4m3fn)


def _bf16(a):
    return np.asarray(a, dtype=np.float32).astype(ml_dtypes.bfloat16)


def kernel(**inputs) -> np.ndarray:
    x_q = np.asarray(inputs["x_q"], np.float32)
    x_kv = np.asarray(inputs["x_kv"], np.float32)
    wq = np.asarray(inputs["wq"], np.float32)
    wkv = np.asarray(inputs["wkv"], np.float32)
    wo = np.asarray(inputs["wo"], np.float32)
    bo = np.asarray(inputs["bo"], np.float32)
    w1 = np.asarray(inputs["w1"], np.float32)
    b1 = np.asarray(inputs["b1"], np.float32)
    w2 = np.asarray(inputs["w2"], np.float32)
    b2 = np.asarray(inputs["b2"], np.float32)
    ln1_g = np.asarray(inputs["ln1_g"], np.float32)
    ln1_b = np.asarray(inputs["ln1_b"], np.float32)
    ln2_g = np.asarray(inputs["ln2_g"], np.float32)
    ln2_b = np.asarray(inputs["ln2_b"], np.float32)

    # Host-side folding of LN affine params into the projection weights.
    shared = {}
    biases = []
    for l in range(L):
        wk_f = wkv[l][:, :E]
        wv_f = wkv[l][:, E:]
        wq_eff = ln1_g[l][:, None] * wq[l]
        wk_eff = ln1_g[l][:, None] * wk_f
        wv_eff = ln1_g[l][:, None] * wv_f
        bq_eff = ln1_b[l] @ wq[l]
        bk_eff = ln1_b[l] @ wk_f
        bv_eff = ln1_b[l] @ wv_f
        bo_eff = bo[l] + bv_eff @ wo[l]
        w1_eff = ln2_g[l][:, None] * w1[l]
        b1_eff = ln2_b[l] @ w1[l] + b1[l]
        biases += [bq_eff, bk_eff, bo_eff, b1_eff, b2[l]]
        shared.update({
            f"wq{l}": _rearr(_fp8(wq_eff * WS), EC),
            f"wk{l}": _rearr(_fp8(wk_eff * WS), EC),
            f"wv{l}": _rearr(_fp8(wv_eff * WS), EC),
            f"wo{l}": _rearr(_bf16(wo[l] / AOS), EC),
            f"w1{l}": _rearr(_fp8(w1_eff * WS), EC),
            f"w2{l}": _rearr(_bf16(w2[l]), MC),
            f"bq{l}": _cols(bq_eff),
            f"bk{l}": _cols(bk_eff),
            f"b1{l}": _cols(b1_eff),
            f"bo{l}": np.ascontiguousarray(np.broadcast_to(bo_eff, (P, E))),
            f"b2{l}": np.ascontiguousarray(np.broadcast_to(b2[l], (P, E))),
        })

    use_bias = any(np.any(b != 0) for b in biases)
    nc = get_nc(use_bias)
    if not use_bias:
        shared = {k: v for k, v in shared.items() if not k.startswith("b")}

    in_maps = []
    for c in range(8):
        b, qc = c // 4, c % 4
        m = dict(shared)
        m["xq"] = np.ascontiguousarray(x_q[b, qc * NQ:(qc + 1) * NQ, :])
        m["xkv"] = np.ascontiguousarray(x_kv[b])
        in_maps.append(m)

    res = bass_utils.run_bass_kernel_spmd(nc, in_maps, core_ids=list(range(8)))

    out = np.empty((2, 2048, E), np.float32)
    for c in range(8):
        b, qc = c // 4, c % 4
        out[b, qc * NQ:(qc + 1) * NQ, :] = res.results[c]["y"]
    return out


# revision 7
# speedup vs baseline: 1.0141x; 1.0141x over previous
"""Trainium2 Bass kernel for a 2-layer cross-encoder (CrossEncoder).

Model: B=2, NQ=NKV=2048, E=512, H=8 (d_head=64), MLP=2048, depth=2, fp32 I/O.

Sharding (8 cores, no collectives): core c handles batch b=c//4 and query
rows [qc*512, (qc+1)*512) with qc=c%4.  Each core computes the full KV
projections for its batch so every core produces its output slice
independently.

Numerics: projection/FFN/attn-value matmuls run in fp8e4m3 with the
DoubleRow perf mode (two 128-deep k-tiles contracted per instruction).
Weights are scaled x32 on the host so their 0.02-std values sit in e4m3's
normal range; every PSUM->SBUF pass descales.  QK^T scores stay bf16.
The residual stream, LayerNorm statistics and softmax normalization stay
fp32.  LN gamma/beta are folded into the projection weights on the host.
The softmax denominator comes free from a ones-column appended to V (the
un-normalized attn@V matmul also computes sum(exp) in column 64); the
attention output is scaled x4 via the replicate row to keep fp8 aoT in
e4m3's normal range.
"""

import numpy as np
import ml_dtypes

import concourse.bass as bass
import concourse.bacc as bacc
import concourse.mybir as mybir
import concourse.tile as tile
from concourse import bass_utils, masks
from contextlib import ExitStack

P = 128
E = 512
EC = E // P        # 4 chunks of the embedding dim
NQ = 512           # query rows per core
QC = NQ // P       # 4 query chunks
NKV = 2048
KC = NKV // P      # 16 key chunks of 128
KN = NKV // 512    # 4 key chunks of 512
H = 8
DH = 64
MLP = 2048
MC = MLP // P      # 16 mlp chunks of 128
L = 2
LN_EPS = 1e-5
F32 = mybir.dt.float32
BF16 = mybir.dt.bfloat16
FP8 = mybir.dt.float8e4
AF = mybir.ActivationFunctionType
ALU = mybir.AluOpType
DR = mybir.MatmulPerfMode.DoubleRow
SCALE = (E // H) ** -0.5
WS = 32.0          # host-side weight scale into fp8
AOS = 4.0          # aoT scale via replicate row

_CACHE = {}


def _build(use_bias):
    """Build the per-core Bass program (identical on all 8 cores)."""
    nc = bacc.Bacc("TRN2", target_bir_lowering=False, debug=False, num_devices=8)

    xq_d = nc.dram_tensor("xq", [NQ, E], F32, kind="ExternalInput").ap()
    xkv_d = nc.dram_tensor("xkv", [NKV, E], F32, kind="ExternalInput").ap()
    wd = []
    for l in range(L):
        wd.append({
            "wq": nc.dram_tensor(f"wq{l}", [P, EC * E], FP8, kind="ExternalInput").ap(),
            "wk": nc.dram_tensor(f"wk{l}", [P, EC * E], FP8, kind="ExternalInput").ap(),
            "wv": nc.dram_tensor(f"wv{l}", [P, EC * E], FP8, kind="ExternalInput").ap(),
            "wo": nc.dram_tensor(f"wo{l}", [P, EC * E], BF16, kind="ExternalInput").ap(),
            "w1": nc.dram_tensor(f"w1{l}", [P, EC * MLP], FP8, kind="ExternalInput").ap(),
            "w2": nc.dram_tensor(f"w2{l}", [P, MC * E], FP8, kind="ExternalInput").ap(),
        })
        if use_bias:
            wd[-1].update({
                "bq": nc.dram_tensor(f"bq{l}", [P, EC], F32, kind="ExternalInput").ap(),
                "bk": nc.dram_tensor(f"bk{l}", [P, EC], F32, kind="ExternalInput").ap(),
                "b1": nc.dram_tensor(f"b1{l}", [P, MC], F32, kind="ExternalInput").ap(),
                "bo": nc.dram_tensor(f"bo{l}", [P, E], F32, kind="ExternalInput").ap(),
                "b2": nc.dram_tensor(f"b2{l}", [P, E], F32, kind="ExternalInput").ap(),
            })
    y_d = nc.dram_tensor("y", [NQ, E], F32, kind="ExternalOutput").ap()

    with tile.TileContext(nc) as tc, ExitStack() as ctx:
        const_pool = ctx.enter_context(tc.tile_pool(name="const", bufs=1))
        ident = const_pool.tile([P, P], BF16)
        masks.make_identity(nc, ident)
        ones_row = const_pool.tile([1, DH], BF16)
        nc.gpsimd.memset(ones_row[:], AOS)
        eps_col = const_pool.tile([P, 1], F32)
        nc.gpsimd.memset(eps_col[:], LN_EPS)

        stats_pool = ctx.enter_context(tc.tile_pool(name="stats", bufs=12))

        def ln_tile(x_t, out_pool, out_name, apply_eng=None):
            """LayerNorm core (x - mu) * rsqrt(var + eps), fp32 in, bf16 out."""
            bnst = stats_pool.tile([P, 6], F32, name="bnst")
            nc.vector.bn_stats(bnst[:], x_t)
            bnag = stats_pool.tile([P, 2], F32, name="bnag")
            nc.vector.bn_aggr(bnag[:], bnst[:])
            sq = stats_pool.tile([P, 1], F32, name="sq")
            nc.scalar.activation(sq[:], bnag[:, 1:2], AF.Sqrt, bias=eps_col[:])
            rstd = stats_pool.tile([P, 1], F32, name="rstd")
            nc.vector.reciprocal(rstd[:], sq[:])
            h_t = out_pool.tile([P, E], BF16, name=out_name, bufs=4)
            (apply_eng or nc.gpsimd).tensor_scalar(
                h_t[:], x_t, bnag[:, 0:1], rstd[:], op0=ALU.subtract, op1=ALU.mult
            )
            return h_t

        # Residual stream: 4 fp32 tiles of [128, 512].
        xq_pool = ctx.enter_context(tc.tile_pool(name="xq", bufs=1))
        xq = []
        for i in range(QC):
            t = xq_pool.tile([P, E], F32, name=f"xq{i}", tag=f"xq{i}")
            nc.sync.dma_start(t[:], xq_d[i * P:(i + 1) * P, :])
            xq.append(t[:])

        # hkv^T: LN1-core of x_kv, transposed to [E, NKV], fp8.  ln1 g/b are
        # folded into the weights, so this is layer-independent.
        hkvT_pool = ctx.enter_context(tc.tile_pool(name="hkvT", bufs=1))
        hkvT = hkvT_pool.tile([P, EC, NKV], FP8, name="hkvT", tag="hkvT")

        # v_aug: per kv-chunk, per head: [64 v-cols | ones | zero pad] fp8.
        vaug_pool = ctx.enter_context(tc.tile_pool(name="vaug", bufs=1))
        v_aug = vaug_pool.tile([P, KC, H, DH + 1], BF16, name="v_aug", tag="v_aug")
        nc.gpsimd.memset(v_aug[:, :, :, DH:DH + 1], 1.0)

        # PSUM pools (8 banks total): pp 2 + ss 2x2 + att 2 = 8.
        pp_pool = ctx.enter_context(tc.tile_pool(name="pp", bufs=2, space="PSUM"))
        ss_pool = ctx.enter_context(tc.tile_pool(name="ss", bufs=2, space="PSUM"))
        att_pool = ctx.enter_context(tc.tile_pool(name="attp", bufs=2, space="PSUM"))

        def transpose_block(dst, src_block, copy_engine="vector"):
            """dst (fp8 SBUF slice) = src_block.T via PE (bf16) + copy."""
            pt = pp_pool.tile([P, E], F32, name="pp", tag="pp")
            ptb = pt[:].bitcast(BF16)[:, 0:P]
            nc.tensor.transpose(ptb, src_block, ident[:])
            if copy_engine == "scalar":
                nc.scalar.copy(dst, ptb)
            else:
                nc.vector.tensor_copy(dst, ptb)

        # Weight pools (bufs=2 -> next layer prefetches during current layer).
        wpool = ctx.enter_context(tc.tile_pool(name="w", bufs=2))

        def alloc_weights_crit(w):
            d = {}
            d["wq"] = wpool.tile([P, EC, E], FP8, name="wq_sb", tag="wq")
            nc.sync.dma_start(d["wq"][:], w["wq"].rearrange("p (c e) -> p c e", c=EC))
            d["wk"] = wpool.tile([P, EC, E], FP8, name="wk_sb", tag="wk")
            nc.sync.dma_start(d["wk"][:], w["wk"].rearrange("p (c e) -> p c e", c=EC))
            d["wv"] = wpool.tile([P, EC, E], FP8, name="wv_sb", tag="wv")
            nc.sync.dma_start(d["wv"][:], w["wv"].rearrange("p (c e) -> p c e", c=EC))
            if use_bias:
                d["bq"] = wpool.tile([P, EC], F32, name="bq_sb", tag="bq")
                nc.sync.dma_start(d["bq"][:], w["bq"])
                d["bk"] = wpool.tile([P, EC], F32, name="bk_sb", tag="bk")
                nc.sync.dma_start(d["bk"][:], w["bk"])
            return d

        def alloc_weights_rest(d, w):
            d["wo"] = wpool.tile([P, EC, E], BF16, name="wo_sb", tag="wo")
            nc.sync.dma_start(d["wo"][:], w["wo"].rearrange("p (c e) -> p c e", c=EC))
            d["w1"] = wpool.tile([P, EC, MLP], FP8, name="w1_sb", tag="w1", bufs=1)
            nc.sync.dma_start(d["w1"][:], w["w1"].rearrange("p (c e) -> p c e", c=EC))
            d["w2"] = wpool.tile([P, MC, E], FP8, name="w2_sb", tag="w2", bufs=1)
            nc.sync.dma_start(d["w2"][:], w["w2"].rearrange("p (c e) -> p c e", c=MC))
            if use_bias:
                d["b1"] = wpool.tile([P, MC], F32, name="b1_sb", tag="b1")
                nc.sync.dma_start(d["b1"][:], w["b1"])
                d["bo"] = wpool.tile([P, E], F32, name="bo_sb", tag="bo", bufs=1)
                nc.sync.dma_start(d["bo"][:], w["bo"])
                d["b2"] = wpool.tile([P, E], F32, name="b2_sb", tag="b2", bufs=1)
                nc.sync.dma_start(d["b2"][:], w["b2"])
            return d

        w0 = None
        with tc.tile_pool(name="xkv", bufs=4) as xkv_pool:
            for ib in range(KC // 2):
                if ib == 2:
                    w0 = alloc_weights_crit(wd[0])
                xkv_t = xkv_pool.tile([P, 2, E], F32, name="xkv_t", tag="xkv_t")
                nc.sync.dma_start(
                    xkv_t[:],
                    xkv_d[ib * 2 * P:(ib + 1) * 2 * P, :].rearrange(
                        "(i p) c -> p i c", p=P
                    ),
                )
                for sub in range(2):
                    i = 2 * ib + sub
                    hkv_t = ln_tile(
                        xkv_t[:, sub, :], xkv_pool, "hkv_t",
                        apply_eng=nc.gpsimd if i % 2 else nc.vector,
                    )
                    for e in range(EC):
                        transpose_block(
                            hkvT[:, e, i * P:(i + 1) * P],
                            hkv_t[:, e * P:(e + 1) * P],
                            "scalar" if (i + e) % 2 else "vector",
                        )

        w0 = alloc_weights_rest(w0, wd[0])

        # Work pools.
        work = ctx.enter_context(tc.tile_pool(name="work", bufs=1))
        big = ctx.enter_context(tc.tile_pool(name="big", bufs=1))
        ex_pool = ctx.enter_context(tc.tile_pool(name="ex", bufs=5))

        def psum_to_sbuf(out, ps, descale, bias, eng=None):
            """out = ps*descale (+bias col) on a vector engine (DVE default)."""
            eng = eng or nc.vector
            if use_bias and bias is not None:
                eng.tensor_scalar(out, ps, descale, bias, op0=ALU.mult, op1=ALU.add)
            else:
                eng.tensor_scalar(out, ps, descale, None, op0=ALU.mult)

        for l in range(L):
            if l == 0:
                wt = w0
            else:
                wt = alloc_weights_crit(wd[l])
                wt = alloc_weights_rest(wt, wd[l])

            # ---- LN1(x_q) and transpose -> hqT [E, NQ] fp8 ----
            hqT = work.tile([P, EC, NQ], FP8, name="hqT", tag="actT")
            for qc in range(QC):
                hq_t = ln_tile(xq[qc], work, "hq_t")
                for e in range(EC):
                    transpose_block(
                        hqT[:, e, qc * P:(qc + 1) * P],
                        hq_t[:, e * P:(e + 1) * P],
                        "scalar" if (qc + e) % 2 else "vector",
                    )

            # ---- q^T = wq^T @ hq^T (+ bq)  [E, NQ] bf16 ----
            qT = [
                work.tile([P, NQ], BF16, name=f"qT{m}", tag=f"qT{m}")
                for m in range(EC)
            ]
            for m in range(EC):
                ps = pp_pool.tile([P, E], F32, name="pp", tag="pp")
                for t in range(2):
                    nc.tensor.matmul(
                        ps[:],
                        wt["wq"][:, 2 * t:2 * t + 2, m * P:(m + 1) * P],
                        hqT[:, 2 * t:2 * t + 2, :],
                        start=(t == 0),
                        stop=(t == 1),
                        perf_mode=DR,
                    )
                psum_to_sbuf(qT[m][:], ps[:], 1.0 / WS,
                             wt["bq"][:, m:m + 1] if use_bias else None)

            # ---- k^T = wk^T @ hkv^T (+ bk)  [E, NKV] bf16 ----
            kT = [
                big.tile([P, NKV], BF16, name=f"kT{m}", tag=f"kT{m}", bufs=2)
                for m in range(EC)
            ]
            for m in range(EC):
                for n in range(KN):
                    ps = pp_pool.tile([P, E], F32, name="pp", tag="pp")
                    for t in range(2):
                        nc.tensor.matmul(
                            ps[:],
                            wt["wk"][:, 2 * t:2 * t + 2, m * P:(m + 1) * P],
                            hkvT[:, 2 * t:2 * t + 2, n * 512:(n + 1) * 512],
                            start=(t == 0),
                            stop=(t == 1),
                            perf_mode=DR,
                        )
                    psum_to_sbuf(kT[m][:, n * 512:(n + 1) * 512], ps[:], 1.0 / WS,
                                 wt["bk"][:, m:m + 1] if use_bias else None)

            # ---- v = hkv @ wv  [NKV, E] -> v_aug fp8 ----
            for m in range(KC):
                ps = pp_pool.tile([P, E], F32, name="pp", tag="pp")
                for t in range(2):
                    nc.tensor.matmul(
                        ps[:],
                        hkvT[:, 2 * t:2 * t + 2, m * P:(m + 1) * P],
                        wt["wv"][:, 2 * t:2 * t + 2, :],
                        start=(t == 0),
                        stop=(t == 1),
                        perf_mode=DR,
                    )
                nc.vector.tensor_scalar(
                    v_aug[:, m, :, 0:DH],
                    ps[:].rearrange("p (h d) -> p h d", h=H),
                    1.0 / WS,
                    None,
                    op0=ALU.mult,
                )

            # ---- attention, head by head; writes aoT (x AOS) fp8 ----
            aoT = work.tile([P, EC, NQ], BF16, name="aoT", tag="aoT")
            for h in range(H):
                fh, r0 = h // 2, (h % 2) * DH
                # rows 0..63 = unnormalized attn@v (x WS); row 64 = sum(exp).
                ps_oT = att_pool.tile([P, E], F32, name="ps_oT", tag="att")
                for g in range(KC // 2):
                    ps_s = ss_pool.tile([P, 2, NQ], F32, name="ps_s", tag="ss")
                    for sub in range(2):
                        m = 2 * g + sub
                        nc.tensor.matmul(
                            ps_s[:, sub, :],
                            kT[fh][r0:r0 + DH, m * P:(m + 1) * P],
                            qT[fh][r0:r0 + DH, :],
                            start=True,
                            stop=True,
                        )
                    ex = ex_pool.tile([P, 2, NQ], BF16, name="ex", tag="ex")
                    nc.scalar.activation(ex[:], ps_s[:], AF.Exp, scale=SCALE)
                    for sub in range(2):
                        m = 2 * g + sub
                        nc.tensor.matmul(
                            ps_oT[0:DH + 1, :],
                            v_aug[:, m, h, :],
                            ex[:, sub, :],
                            start=(m == 0),
                            stop=(m == KC - 1),
                        )
                # normalize: aoT rows = AOS * unnorm / denom.
                rcp = stats_pool.tile([1, NQ], BF16, name="rcp", bufs=2)
                with nc.allow_low_precision(reason="f32r recip row for PE replicate"):
                    nc.vector.reciprocal(rcp[:], ps_oT[DH:DH + 1, :])
                ps_rep = pp_pool.tile([P, E], F32, name="pp", tag="pp")
                nc.tensor.matmul(
                    ps_rep[0:DH, :],
                    ones_row[:],
                    rcp[:],
                    start=True,
                    stop=True,
                )
                u_sb = work.tile([P, NQ], BF16, name="u_sb", bufs=2)
                nc.vector.tensor_copy(u_sb[0:DH, :], ps_oT[0:DH, :])
                nc.vector.tensor_mul(
                    aoT[r0:r0 + DH, fh, :], u_sb[0:DH, :], ps_rep[0:DH, :]
                )

            # ---- out-proj, residual ----
            for qc in range(QC):
                ps = pp_pool.tile([P, E], F32, name="pp", tag="pp")
                for kk in range(EC):
                    nc.tensor.matmul(
                        ps[:],
                        aoT[:, kk, qc * P:(qc + 1) * P],
                        wt["wo"][:, kk, :],
                        start=(kk == 0),
                        stop=(kk == EC - 1),
                    )
                if use_bias:
                    nc.vector.tensor_add(ps[:], ps[:], wt["bo"][:])
                nc.vector.tensor_add(xq[qc], xq[qc], ps[:])

            # ---- LN2 + transpose -> h2T fp8 ----
            h2T = work.tile([P, EC, NQ], FP8, name="h2T", tag="actT2")
            for qc in range(QC):
                h2_t = ln_tile(xq[qc], work, "hq_t")
                for e in range(EC):
                    transpose_block(
                        h2T[:, e, qc * P:(qc + 1) * P],
                        h2_t[:, e * P:(e + 1) * P],
                        "scalar" if (qc + e) % 2 else "vector",
                    )

            # ---- FFN1: g^T = gelu((w1^T @ h2^T)/WS + b1)  [MLP, NQ] fp8 ----
            gT = big.tile([P, MC, NQ], FP8, name="gT", tag="gT")
            for mg in range(MC // 2):
                ps = ss_pool.tile([P, 2, NQ], F32, name="ps_f", tag="ss")
                for j in range(2):
                    m = 2 * mg + j
                    for t in range(2):
                        nc.tensor.matmul(
                            ps[:, j, :],
                            wt["w1"][:, 2 * t:2 * t + 2, m * P:(m + 1) * P],
                            h2T[:, 2 * t:2 * t + 2, :],
                            start=(t == 0),
                            stop=(t == 1),
                            perf_mode=DR,
                        )
                if use_bias:
                    for j in range(2):
                        m = 2 * mg + j
                        nc.scalar.activation(
                            gT[:, m, :], ps[:, j, :], AF.Gelu,
                            bias=wt["b1"][:, m:m + 1], scale=1.0 / WS,
                        )
                else:
                    nc.scalar.activation(
                        gT[:, 2 * mg:2 * mg + 2, :], ps[:], AF.Gelu, scale=1.0 / WS
                    )

            # ---- FFN2 + residual ----
            for qc in range(QC):
                ps = pp_pool.tile([P, E], F32, name="pp", tag="pp")
                for g in range(MC // 2):
                    nc.tensor.matmul(
                        ps[:],
                        gT[:, 2 * g:2 * g + 2, qc * P:(qc + 1) * P],
                        wt["w2"][:, 2 * g:2 * g + 2, :],
                        start=(g == 0),
                        stop=(g == MC // 2 - 1),
                        perf_mode=DR,
                    )
                nc.vector.tensor_scalar(ps[:], ps[:], 1.0 / WS, None, op0=ALU.mult)
                if use_bias:
                    nc.vector.tensor_add(ps[:], ps[:], wt["b2"][:])
                nc.vector.tensor_add(xq[qc], xq[qc], ps[:])

        for qc in range(QC):
            nc.sync.dma_start(y_d[qc * P:(qc + 1) * P, :], xq[qc])

    nc.compile()
    return nc


def get_nc(use_bias=False):
    key = ("nc", use_bias)
    if key not in _CACHE:
        _CACHE[key] = _build(use_bias)
    return _CACHE[key]


def _rearr(w, k):
    """[k*128, C] row-major -> [128, k*C] with free layout (chunk, col)."""
    c = w.shape[1]
    return np.ascontiguousarray(
        w.reshape(k, P, c).transpose(1, 0, 2).reshape(P, k * c)
    )


def _cols(v):
    """[k*128] -> [128, k]: column m holds v[m*128:(m+1)*128]."""
    k = v.shape[0] // P
    return np.ascontiguousarray(v.reshape(k, P).T)


def _fp8(a):
    return np.asarray(a, dtype=np.float32).astype(ml_dtypes.float8_e4m3fn)


def _bf16(a):
    return np.asarray(a, dtype=np.float32).astype(ml_dtypes.bfloat16)


def kernel(**inputs) -> np.ndarray:
    x_q = np.asarray(inputs["x_q"], np.float32)
    x_kv = np.asarray(inputs["x_kv"], np.float32)
    wq = np.asarray(inputs["wq"], np.float32)
    wkv = np.asarray(inputs["wkv"], np.float32)
    wo = np.asarray(inputs["wo"], np.float32)
    bo = np.asarray(inputs["bo"], np.float32)
    w1 = np.asarray(inputs["w1"], np.float32)
    b1 = np.asarray(inputs["b1"], np.float32)
    w2 = np.asarray(inputs["w2"], np.float32)
    b2 = np.asarray(inputs["b2"], np.float32)
    ln1_g = np.asarray(inputs["ln1_g"], np.float32)
    ln1_b = np.asarray(inputs["ln1_b"], np.float32)
    ln2_g = np.asarray(inputs["ln2_g"], np.float32)
    ln2_b = np.asarray(inputs["ln2_b"], np.float32)

    # Host-side folding of LN affine params into the projection weights.
    shared = {}
    biases = []
    for l in range(L):
        wk_f = wkv[l][:, :E]
        wv_f = wkv[l][:, E:]
        wq_eff = ln1_g[l][:, None] * wq[l]
        wk_eff = ln1_g[l][:, None] * wk_f
        wv_eff = ln1_g[l][:, None] * wv_f
        bq_eff = ln1_b[l] @ wq[l]
        bk_eff = ln1_b[l] @ wk_f
        bv_eff = ln1_b[l] @ wv_f
        bo_eff = bo[l] + bv_eff @ wo[l]
        w1_eff = ln2_g[l][:, None] * w1[l]
        b1_eff = ln2_b[l] @ w1[l] + b1[l]
        biases += [bq_eff, bk_eff, bo_eff, b1_eff, b2[l]]
        shared.update({
            f"wq{l}": _rearr(_fp8(wq_eff * WS), EC),
            f"wk{l}": _rearr(_fp8(wk_eff * WS), EC),
            f"wv{l}": _rearr(_fp8(wv_eff * WS), EC),
            f"wo{l}": _rearr(_bf16(wo[l] / AOS), EC),
            f"w1{l}": _rearr(_fp8(w1_eff * WS), EC),
            f"w2{l}": _rearr(_fp8(w2[l] * WS), MC),
            f"bq{l}": _cols(bq_eff),
            f"bk{l}": _cols(bk_eff),
            f"b1{l}": _cols(b1_eff),
            f"bo{l}": np.ascontiguousarray(np.broadcast_to(bo_eff, (P, E))),
            f"b2{l}": np.ascontiguousarray(np.broadcast_to(b2[l], (P, E))),
        })

    use_bias = any(np.any(b != 0) for b in biases)
    nc = get_nc(use_bias)
    if not use_bias:
        shared = {k: v for k, v in shared.items() if not k.startswith("b")}

    in_maps = []
    for c in range(8):
        b, qc = c // 4, c % 4
        m = dict(shared)
        m["xq"] = np.ascontiguousarray(x_q[b, qc * NQ:(qc + 1) * NQ, :])
        m["xkv"] = np.ascontiguousarray(x_kv[b])
        in_maps.append(m)

    res = bass_utils.run_bass_kernel_spmd(nc, in_maps, core_ids=list(range(8)))

    out = np.empty((2, 2048, E), np.float32)
    for c in range(8):
        b, qc = c // 4, c % 4
        out[b, qc * NQ:(qc + 1) * NQ, :] = res.results[c]["y"]
    return out


# revision 8
# speedup vs baseline: 1.0298x; 1.0155x over previous
"""Trainium2 Bass kernel for a 2-layer cross-encoder (CrossEncoder).

Model: B=2, NQ=NKV=2048, E=512, H=8 (d_head=64), MLP=2048, depth=2, fp32 I/O.

Sharding (8 cores, no collectives): core c handles batch b=c//4 and query
rows [qc*512, (qc+1)*512) with qc=c%4.  Each core computes the full KV
projections for its batch so every core produces its output slice
independently.

Numerics: projection/FFN/attn-value matmuls run in fp8e4m3 with the
DoubleRow perf mode (two 128-deep k-tiles contracted per instruction).
Weights are scaled x32 on the host so their 0.02-std values sit in e4m3's
normal range; every PSUM->SBUF pass descales.  QK^T scores stay bf16.
The residual stream, LayerNorm statistics and softmax normalization stay
fp32.  LN gamma/beta are folded into the projection weights on the host.
The softmax denominator comes free from a ones-column appended to V (the
un-normalized attn@V matmul also computes sum(exp) in column 64); the
attention output is scaled x4 via the replicate row to keep fp8 aoT in
e4m3's normal range.
"""

import numpy as np
import ml_dtypes

import concourse.bass as bass
import concourse.bacc as bacc
import concourse.mybir as mybir
import concourse.tile as tile
from concourse import bass_utils, masks
from contextlib import ExitStack

P = 128
E = 512
EC = E // P        # 4 chunks of the embedding dim
NQ = 512           # query rows per core
QC = NQ // P       # 4 query chunks
NKV = 2048
KC = NKV // P      # 16 key chunks of 128
KN = NKV // 512    # 4 key chunks of 512
H = 8
DH = 64
MLP = 2048
MC = MLP // P      # 16 mlp chunks of 128
L = 2
LN_EPS = 1e-5
F32 = mybir.dt.float32
BF16 = mybir.dt.bfloat16
FP8 = mybir.dt.float8e4
AF = mybir.ActivationFunctionType
ALU = mybir.AluOpType
DR = mybir.MatmulPerfMode.DoubleRow
SCALE = (E // H) ** -0.5
WS = 32.0          # host-side weight scale into fp8
AOS = 4.0          # aoT scale via replicate row

_CACHE = {}


def _build(use_bias):
    """Build the per-core Bass program (identical on all 8 cores)."""
    nc = bacc.Bacc("TRN2", target_bir_lowering=False, debug=False, num_devices=8)

    xq_d = nc.dram_tensor("xq", [NQ, E], F32, kind="ExternalInput").ap()
    xkv_d = nc.dram_tensor("xkv", [NKV, E], F32, kind="ExternalInput").ap()
    wd = []
    for l in range(L):
        wd.append({
            "wq": nc.dram_tensor(f"wq{l}", [P, EC * E], FP8, kind="ExternalInput").ap(),
            "wk": nc.dram_tensor(f"wk{l}", [P, EC * E], FP8, kind="ExternalInput").ap(),
            "wv": nc.dram_tensor(f"wv{l}", [P, EC * E], FP8, kind="ExternalInput").ap(),
            "wo": nc.dram_tensor(f"wo{l}", [P, EC * E], FP8, kind="ExternalInput").ap(),
            "w1": nc.dram_tensor(f"w1{l}", [P, EC * MLP], FP8, kind="ExternalInput").ap(),
            "w2": nc.dram_tensor(f"w2{l}", [P, MC * E], BF16, kind="ExternalInput").ap(),
        })
        if use_bias:
            wd[-1].update({
                "bq": nc.dram_tensor(f"bq{l}", [P, EC], F32, kind="ExternalInput").ap(),
                "bk": nc.dram_tensor(f"bk{l}", [P, EC], F32, kind="ExternalInput").ap(),
                "b1": nc.dram_tensor(f"b1{l}", [P, MC], F32, kind="ExternalInput").ap(),
                "bo": nc.dram_tensor(f"bo{l}", [P, E], F32, kind="ExternalInput").ap(),
                "b2": nc.dram_tensor(f"b2{l}", [P, E], F32, kind="ExternalInput").ap(),
            })
    y_d = nc.dram_tensor("y", [NQ, E], F32, kind="ExternalOutput").ap()

    with tile.TileContext(nc) as tc, ExitStack() as ctx:
        const_pool = ctx.enter_context(tc.tile_pool(name="const", bufs=1))
        ident = const_pool.tile([P, P], BF16)
        masks.make_identity(nc, ident)
        ones_row = const_pool.tile([1, DH], BF16)
        nc.gpsimd.memset(ones_row[:], AOS)
        eps_col = const_pool.tile([P, 1], F32)
        nc.gpsimd.memset(eps_col[:], LN_EPS)

        stats_pool = ctx.enter_context(tc.tile_pool(name="stats", bufs=12))

        def ln_tile(x_t, out_pool, out_name, apply_eng=None):
            """LayerNorm core (x - mu) * rsqrt(var + eps), fp32 in, bf16 out."""
            bnst = stats_pool.tile([P, 6], F32, name="bnst")
            nc.vector.bn_stats(bnst[:], x_t)
            bnag = stats_pool.tile([P, 2], F32, name="bnag")
            nc.vector.bn_aggr(bnag[:], bnst[:])
            sq = stats_pool.tile([P, 1], F32, name="sq")
            nc.scalar.activation(sq[:], bnag[:, 1:2], AF.Sqrt, bias=eps_col[:])
            rstd = stats_pool.tile([P, 1], F32, name="rstd")
            nc.vector.reciprocal(rstd[:], sq[:])
            h_t = out_pool.tile([P, E], BF16, name=out_name, bufs=4)
            (apply_eng or nc.gpsimd).tensor_scalar(
                h_t[:], x_t, bnag[:, 0:1], rstd[:], op0=ALU.subtract, op1=ALU.mult
            )
            return h_t

        # Residual stream: 4 fp32 tiles of [128, 512].
        xq_pool = ctx.enter_context(tc.tile_pool(name="xq", bufs=1))
        xq = []
        for i in range(QC):
            t = xq_pool.tile([P, E], F32, name=f"xq{i}", tag=f"xq{i}")
            nc.sync.dma_start(t[:], xq_d[i * P:(i + 1) * P, :])
            xq.append(t[:])

        # hkv^T: LN1-core of x_kv, transposed to [E, NKV], fp8.  ln1 g/b are
        # folded into the weights, so this is layer-independent.
        hkvT_pool = ctx.enter_context(tc.tile_pool(name="hkvT", bufs=1))
        hkvT = hkvT_pool.tile([P, EC, NKV], FP8, name="hkvT", tag="hkvT")

        # v_aug: per kv-chunk, per head: [64 v-cols | ones | zero pad] fp8.
        vaug_pool = ctx.enter_context(tc.tile_pool(name="vaug", bufs=1))
        v_aug = vaug_pool.tile([P, KC, H, P], FP8, name="v_aug", tag="v_aug")
        nc.gpsimd.memset(v_aug[:, :, :, DH:], 0.0)
        nc.gpsimd.memset(v_aug[:, :, :, DH:DH + 1], 1.0)

        # PSUM pools (8 banks total): pp 2 + ss 2x2 + att 2 = 8.
        pp_pool = ctx.enter_context(tc.tile_pool(name="pp", bufs=2, space="PSUM"))
        ss_pool = ctx.enter_context(tc.tile_pool(name="ss", bufs=2, space="PSUM"))
        att_pool = ctx.enter_context(tc.tile_pool(name="attp", bufs=2, space="PSUM"))

        def transpose_block(dst, src_block, copy_engine="vector"):
            """dst (fp8 SBUF slice) = src_block.T via PE (bf16) + copy."""
            pt = pp_pool.tile([P, E], F32, name="pp", tag="pp")
            ptb = pt[:].bitcast(BF16)[:, 0:P]
            nc.tensor.transpose(ptb, src_block, ident[:])
            if copy_engine == "scalar":
                nc.scalar.copy(dst, ptb)
            else:
                nc.vector.tensor_copy(dst, ptb)

        # Weight pools (bufs=2 -> next layer prefetches during current layer).
        wpool = ctx.enter_context(tc.tile_pool(name="w", bufs=2))

        def alloc_weights_crit(w):
            d = {}
            d["wq"] = wpool.tile([P, EC, E], FP8, name="wq_sb", tag="wq")
            nc.sync.dma_start(d["wq"][:], w["wq"].rearrange("p (c e) -> p c e", c=EC))
            d["wk"] = wpool.tile([P, EC, E], FP8, name="wk_sb", tag="wk")
            nc.sync.dma_start(d["wk"][:], w["wk"].rearrange("p (c e) -> p c e", c=EC))
            d["wv"] = wpool.tile([P, EC, E], FP8, name="wv_sb", tag="wv")
            nc.sync.dma_start(d["wv"][:], w["wv"].rearrange("p (c e) -> p c e", c=EC))
            if use_bias:
                d["bq"] = wpool.tile([P, EC], F32, name="bq_sb", tag="bq")
                nc.sync.dma_start(d["bq"][:], w["bq"])
                d["bk"] = wpool.tile([P, EC], F32, name="bk_sb", tag="bk")
                nc.sync.dma_start(d["bk"][:], w["bk"])
            return d

        def alloc_weights_rest(d, w):
            d["wo"] = wpool.tile([P, EC, E], FP8, name="wo_sb", tag="wo")
            nc.sync.dma_start(d["wo"][:], w["wo"].rearrange("p (c e) -> p c e", c=EC))
            d["w1"] = wpool.tile([P, EC, MLP], FP8, name="w1_sb", tag="w1", bufs=1)
            nc.sync.dma_start(d["w1"][:], w["w1"].rearrange("p (c e) -> p c e", c=EC))
            d["w2"] = wpool.tile([P, MC, E], BF16, name="w2_sb", tag="w2", bufs=1)
            nc.sync.dma_start(d["w2"][:], w["w2"].rearrange("p (c e) -> p c e", c=MC))
            if use_bias:
                d["b1"] = wpool.tile([P, MC], F32, name="b1_sb", tag="b1")
                nc.sync.dma_start(d["b1"][:], w["b1"])
                d["bo"] = wpool.tile([P, E], F32, name="bo_sb", tag="bo", bufs=1)
                nc.sync.dma_start(d["bo"][:], w["bo"])
                d["b2"] = wpool.tile([P, E], F32, name="b2_sb", tag="b2", bufs=1)
                nc.sync.dma_start(d["b2"][:], w["b2"])
            return d

        w0 = None
        with tc.tile_pool(name="xkv", bufs=4) as xkv_pool:
            for ib in range(KC // 2):
                if ib == 2:
                    w0 = alloc_weights_crit(wd[0])
                xkv_t = xkv_pool.tile([P, 2, E], F32, name="xkv_t", tag="xkv_t")
                nc.sync.dma_start(
                    xkv_t[:],
                    xkv_d[ib * 2 * P:(ib + 1) * 2 * P, :].rearrange(
                        "(i p) c -> p i c", p=P
                    ),
                )
                for sub in range(2):
                    i = 2 * ib + sub
                    hkv_t = ln_tile(
                        xkv_t[:, sub, :], xkv_pool, "hkv_t",
                        apply_eng=nc.gpsimd if i % 2 else nc.vector,
                    )
                    for e in range(EC):
                        transpose_block(
                            hkvT[:, e, i * P:(i + 1) * P],
                            hkv_t[:, e * P:(e + 1) * P],
                            "scalar" if (i + e) % 2 else "vector",
                        )

        w0 = alloc_weights_rest(w0, wd[0])

        # Work pools.
        work = ctx.enter_context(tc.tile_pool(name="work", bufs=1))
        big = ctx.enter_context(tc.tile_pool(name="big", bufs=1))
        ex_pool = ctx.enter_context(tc.tile_pool(name="ex", bufs=5))

        def psum_to_sbuf(out, ps, descale, bias, eng=None):
            """out = ps*descale (+bias col) on a vector engine (DVE default)."""
            eng = eng or nc.vector
            if use_bias and bias is not None:
                eng.tensor_scalar(out, ps, descale, bias, op0=ALU.mult, op1=ALU.add)
            else:
                eng.tensor_scalar(out, ps, descale, None, op0=ALU.mult)

        for l in range(L):
            if l == 0:
                wt = w0
            else:
                wt = alloc_weights_crit(wd[l])
                wt = alloc_weights_rest(wt, wd[l])

            # ---- LN1(x_q) and transpose -> hqT [E, NQ] fp8 ----
            hqT = work.tile([P, EC, NQ], FP8, name="hqT", tag="actT")
            for qc in range(QC):
                hq_t = ln_tile(xq[qc], work, "hq_t")
                for e in range(EC):
                    transpose_block(
                        hqT[:, e, qc * P:(qc + 1) * P],
                        hq_t[:, e * P:(e + 1) * P],
                        "scalar" if (qc + e) % 2 else "vector",
                    )

            # ---- q^T = wq^T @ hq^T (+ bq)  [E, NQ] bf16 ----
            qT = [
                work.tile([P, NQ], BF16, name=f"qT{m}", tag=f"qT{m}")
                for m in range(EC)
            ]
            for m in range(EC):
                ps = pp_pool.tile([P, E], F32, name="pp", tag="pp")
                for t in range(2):
                    nc.tensor.matmul(
                        ps[:],
                        wt["wq"][:, 2 * t:2 * t + 2, m * P:(m + 1) * P],
                        hqT[:, 2 * t:2 * t + 2, :],
                        start=(t == 0),
                        stop=(t == 1),
                        perf_mode=DR,
                    )
                psum_to_sbuf(qT[m][:], ps[:], 1.0 / WS,
                             wt["bq"][:, m:m + 1] if use_bias else None)

            # ---- k^T = wk^T @ hkv^T (+ bk)  [E, NKV] bf16 ----
            kT = [
                big.tile([P, NKV], BF16, name=f"kT{m}", tag=f"kT{m}", bufs=2)
                for m in range(EC)
            ]
            for m in range(EC):
                for n in range(KN):
                    ps = pp_pool.tile([P, E], F32, name="pp", tag="pp")
                    for t in range(2):
                        nc.tensor.matmul(
                            ps[:],
                            wt["wk"][:, 2 * t:2 * t + 2, m * P:(m + 1) * P],
                            hkvT[:, 2 * t:2 * t + 2, n * 512:(n + 1) * 512],
                            start=(t == 0),
                            stop=(t == 1),
                            perf_mode=DR,
                        )
                    psum_to_sbuf(kT[m][:, n * 512:(n + 1) * 512], ps[:], 1.0 / WS,
                                 wt["bk"][:, m:m + 1] if use_bias else None)

            # ---- v = hkv @ wv  [NKV, E] -> v_aug fp8 ----
            for m in range(KC):
                ps = pp_pool.tile([P, E], F32, name="pp", tag="pp")
                for t in range(2):
                    nc.tensor.matmul(
                        ps[:],
                        hkvT[:, 2 * t:2 * t + 2, m * P:(m + 1) * P],
                        wt["wv"][:, 2 * t:2 * t + 2, :],
                        start=(t == 0),
                        stop=(t == 1),
                        perf_mode=DR,
                    )
                nc.vector.tensor_scalar(
                    v_aug[:, m, :, 0:DH],
                    ps[:].rearrange("p (h d) -> p h d", h=H),
                    1.0 / WS,
                    None,
                    op0=ALU.mult,
                )

            # ---- attention, head by head; writes aoT (x AOS) fp8 ----
            aoT = work.tile([P, EC, NQ], FP8, name="aoT", tag="aoT")
            for h in range(H):
                fh, r0 = h // 2, (h % 2) * DH
                # rows 0..63 = unnormalized attn@v (x WS); row 64 = sum(exp).
                ps_oT = att_pool.tile([P, E], F32, name="ps_oT", tag="att")
                for g in range(KC // 2):
                    ps_s = ss_pool.tile([P, 2, NQ], F32, name="ps_s", tag="ss")
                    for sub in range(2):
                        m = 2 * g + sub
                        nc.tensor.matmul(
                            ps_s[:, sub, :],
                            kT[fh][r0:r0 + DH, m * P:(m + 1) * P],
                            qT[fh][r0:r0 + DH, :],
                            start=True,
                            stop=True,
                        )
                    ex = ex_pool.tile([P, 2, NQ], FP8, name="ex", tag="ex")
                    nc.scalar.activation(ex[:], ps_s[:], AF.Exp, scale=SCALE)
                    nc.tensor.matmul(
                        ps_oT[:],
                        v_aug[:, 2 * g:2 * g + 2, h, :],
                        ex[:],
                        start=(g == 0),
                        stop=(g == KC // 2 - 1),
                        perf_mode=DR,
                    )
                # normalize: aoT rows = AOS * unnorm / denom.
                rcp = stats_pool.tile([1, NQ], BF16, name="rcp", bufs=2)
                with nc.allow_low_precision(reason="f32r recip row for PE replicate"):
                    nc.vector.reciprocal(rcp[:], ps_oT[DH:DH + 1, :])
                ps_rep = pp_pool.tile([P, E], F32, name="pp", tag="pp")
                nc.tensor.matmul(
                    ps_rep[0:DH, :],
                    ones_row[:],
                    rcp[:],
                    start=True,
                    stop=True,
                )
                u_sb = work.tile([P, NQ], BF16, name="u_sb", bufs=2)
                nc.vector.tensor_copy(u_sb[0:DH, :], ps_oT[0:DH, :])
                nc.vector.tensor_mul(
                    aoT[r0:r0 + DH, fh, :], u_sb[0:DH, :], ps_rep[0:DH, :]
                )

            # ---- out-proj, residual ----
            for qc in range(QC):
                ps = pp_pool.tile([P, E], F32, name="pp", tag="pp")
                for t in range(2):
                    nc.tensor.matmul(
                        ps[:],
                        aoT[:, 2 * t:2 * t + 2, qc * P:(qc + 1) * P],
                        wt["wo"][:, 2 * t:2 * t + 2, :],
                        start=(t == 0),
                        stop=(t == 1),
                        perf_mode=DR,
                    )
                nc.vector.tensor_scalar(ps[:], ps[:], 1.0 / (WS * AOS), None, op0=ALU.mult)
                if use_bias:
                    nc.vector.tensor_add(ps[:], ps[:], wt["bo"][:])
                nc.vector.tensor_add(xq[qc], xq[qc], ps[:])

            # ---- LN2 + transpose -> h2T fp8 ----
            h2T = work.tile([P, EC, NQ], FP8, name="h2T", tag="actT2")
            for qc in range(QC):
                h2_t = ln_tile(xq[qc], work, "hq_t")
                for e in range(EC):
                    transpose_block(
                        h2T[:, e, qc * P:(qc + 1) * P],
                        h2_t[:, e * P:(e + 1) * P],
                        "scalar" if (qc + e) % 2 else "vector",
                    )

            # ---- FFN1: g^T = gelu((w1^T @ h2^T)/WS + b1)  [MLP, NQ] fp8 ----
            gT = big.tile([P, MC, NQ], BF16, name="gT", tag="gT")
            for mg in range(MC // 2):
                ps = ss_pool.tile([P, 2, NQ], F32, name="ps_f", tag="ss")
                for j in range(2):
                    m = 2 * mg + j
                    for t in range(2):
                        nc.tensor.matmul(
                            ps[:, j, :],
                            wt["w1"][:, 2 * t:2 * t + 2, m * P:(m + 1) * P],
                            h2T[:, 2 * t:2 * t + 2, :],
                            start=(t == 0),
                            stop=(t == 1),
                            perf_mode=DR,
                        )
                if use_bias:
                    for j in range(2):
                        m = 2 * mg + j
                        nc.scalar.activation(
                            gT[:, m, :], ps[:, j, :], AF.Gelu,
                            bias=wt["b1"][:, m:m + 1], scale=1.0 / WS,
                        )
                else:
                    nc.scalar.activation(
                        gT[:, 2 * mg:2 * mg + 2, :], ps[:], AF.Gelu, scale=1.0 / WS
                    )

            # ---- FFN2 + residual ----
            for qc in range(QC):
                ps = pp_pool.tile([P, E], F32, name="pp", tag="pp")
                for g in range(MC):
                    nc.tensor.matmul(
                        ps[:],
                        gT[:, g, qc * P:(qc + 1) * P],
                        wt["w2"][:, g, :],
                        start=(g == 0),
                        stop=(g == MC - 1),
                    )
                if use_bias:
                    nc.vector.tensor_add(ps[:], ps[:], wt["b2"][:])
                nc.vector.tensor_add(xq[qc], xq[qc], ps[:])

        for qc in range(QC):
            nc.sync.dma_start(y_d[qc * P:(qc + 1) * P, :], xq[qc])

    nc.compile()
    return nc


def get_nc(use_bias=False):
    key = ("nc", use_bias)
    if key not in _CACHE:
        _CACHE[key] = _build(use_bias)
    return _CACHE[key]


def _rearr(w, k):
    """[k*128, C] row-major -> [128, k*C] with free layout (chunk, col)."""
    c = w.shape[1]
    return np.ascontiguousarray(
        w.reshape(k, P, c).transpose(1, 0, 2).reshape(P, k * c)
    )


def _cols(v):
    """[k*128] -> [128, k]: column m holds v[m*128:(m+1)*128]."""
    k = v.shape[0] // P
    return np.ascontiguousarray(v.reshape(k, P).T)


def _fp8(a):
    return np.asarray(a, dtype=np.float32).astype(ml_dtypes.float8_e4m3fn)


def _bf16(a):
    return np.asarray(a, dtype=np.float32).astype(ml_dtypes.bfloat16)


def kernel(**inputs) -> np.ndarray:
    x_q = np.asarray(inputs["x_q"], np.float32)
    x_kv = np.asarray(inputs["x_kv"], np.float32)
    wq = np.asarray(inputs["wq"], np.float32)
    wkv = np.asarray(inputs["wkv"], np.float32)
    wo = np.asarray(inputs["wo"], np.float32)
    bo = np.asarray(inputs["bo"], np.float32)
    w1 = np.asarray(inputs["w1"], np.float32)
    b1 = np.asarray(inputs["b1"], np.float32)
    w2 = np.asarray(inputs["w2"], np.float32)
    b2 = np.asarray(inputs["b2"], np.float32)
    ln1_g = np.asarray(inputs["ln1_g"], np.float32)
    ln1_b = np.asarray(inputs["ln1_b"], np.float32)
    ln2_g = np.asarray(inputs["ln2_g"], np.float32)
    ln2_b = np.asarray(inputs["ln2_b"], np.float32)

    # Host-side folding of LN affine params into the projection weights.
    shared = {}
    biases = []
    for l in range(L):
        wk_f = wkv[l][:, :E]
        wv_f = wkv[l][:, E:]
        wq_eff = ln1_g[l][:, None] * wq[l]
        wk_eff = ln1_g[l][:, None] * wk_f
        wv_eff = ln1_g[l][:, None] * wv_f
        bq_eff = ln1_b[l] @ wq[l]
        bk_eff = ln1_b[l] @ wk_f
        bv_eff = ln1_b[l] @ wv_f
        bo_eff = bo[l] + bv_eff @ wo[l]
        w1_eff = ln2_g[l][:, None] * w1[l]
        b1_eff = ln2_b[l] @ w1[l] + b1[l]
        biases += [bq_eff, bk_eff, bo_eff, b1_eff, b2[l]]
        shared.update({
            f"wq{l}": _rearr(_fp8(wq_eff * WS), EC),
            f"wk{l}": _rearr(_fp8(wk_eff * WS), EC),
            f"wv{l}": _rearr(_fp8(wv_eff * WS), EC),
            f"wo{l}": _rearr(_fp8(wo[l] * WS), EC),
            f"w1{l}": _rearr(_fp8(w1_eff * WS), EC),
            f"w2{l}": _rearr(_bf16(w2[l]), MC),
            f"bq{l}": _cols(bq_eff),
            f"bk{l}": _cols(bk_eff),
            f"b1{l}": _cols(b1_eff),
            f"bo{l}": np.ascontiguousarray(np.broadcast_to(bo_eff, (P, E))),
            f"b2{l}": np.ascontiguousarray(np.broadcast_to(b2[l], (P, E))),
        })

    use_bias = any(np.any(b != 0) for b in biases)
    nc = get_nc(use_bias)
    if not use_bias:
        shared = {k: v for k, v in shared.items() if not k.startswith("b")}

    in_maps = []
    for c in range(8):
        b, qc = c // 4, c % 4
        m = dict(shared)
        m["xq"] = np.ascontiguousarray(x_q[b, qc * NQ:(qc + 1) * NQ, :])
        m["xkv"] = np.ascontiguousarray(x_kv[b])
        in_maps.append(m)

    res = bass_utils.run_bass_kernel_spmd(nc, in_maps, core_ids=list(range(8)))

    out = np.empty((2, 2048, E), np.float32)
    for c in range(8):
        b, qc = c // 4, c % 4
        out[b, qc * NQ:(qc + 1) * NQ, :] = res.results[c]["y"]
    return out
